# revision 1
# baseline (speedup 1.0000x reference)
"""Trainium2 Bass kernel for nn_DeepRecursiveNetwork.

Math (reference): 30 outer steps; each step, per block n (0..9):
    inp  = h[n] + block_in[n]           (block_in = x_emb for n=0 else h[n-1] from prev step)
    inner equilibrium, 5 iters from h'=0:
        h' = 0.5 h' + 0.5 tanh(h' @ W[n].T + b[n] + inp)
    h[n] = 0.5 h[n] + 0.5 h'
Output: h[9] @ head_W.T + head_b.

Device formulation (per core, 8-way data parallel over batch, B_local=128):
  - All recurrent tensors live TRANSPOSED in SBUF as [128, 8*128] tiles laid
    out (d_lo, (d_hi, b)) so matmuls (out = lhsT.T @ rhs, contraction on the
    partition dim) need no transposes anywhere.
  - Inner state substitution u = 2*h' with pre-halved weights Wt = W.T/2:
        u_{j+1} = 0.5*u_j + tanh(Wt.T-matmul(u_j) + c + b[n])
    (one fused scalar_tensor_tensor per tile), u_1 = tanh(c + b[n]),
    h'_5 = u_5/2, outer update v[n] = 0.5 v[n] + 0.25 u_5.
  - Blocks processed in reverse order per step so block n reads the
    previous-step value of v[n-1] with pure in-place updates.
  - Matmuls in fp16 (full PE rate); two complementary fp16 roundings of the
    weights are used on alternating outer steps so the correlated rounding
    bias cancels (measured ~0.5% max err vs 1.0% with plain fp16 rounding).
    All elementwise math is fp32; PSUM accumulation is fp32.
  - Weights (2 x 20 MB fp16) stream from HBM per (block, step), double
    buffered; 2 MB per block-step against ~20 us of PE work.
  - Blocks are processed as software-pipelined pairs (n, n-1): within a step
    they are data-independent, so their matmul iterations interleave on the
    PE and each block's psum->add->tanh->axpy chain hides under the other
    block's 64-matmul group instead of stalling the PE.
"""

import numpy as np

import concourse.bass as bass
import concourse.bacc as bacc
import concourse.mybir as mybir
from concourse.bass_utils import run_bass_kernel_spmd
from concourse.tile import TileContext

F32 = mybir.dt.float32
F16 = mybir.dt.float16

B, DIN, H, DOUT, NB = 1024, 512, 1024, 512, 10
NCORES = 8
BL = B // NCORES  # 128 batch per core
KH = H // 128     # 8 k/m tiles over H
KD = DIN // 128   # 4 k tiles over DIN
KO = DOUT // 128  # 4 m tiles over DOUT
INNER = 5
Tanh = mybir.ActivationFunctionType.Tanh
Ident = mybir.ActivationFunctionType.Identity
MULT = mybir.AluOpType.mult
ADD = mybir.AluOpType.add


def build_nc(steps: int):
    nc = bacc.Bacc(None, target_bir_lowering=False)
    xT = nc.dram_tensor("xT", [128, KD * BL], F32, kind="ExternalInput")
    embWT = nc.dram_tensor("embWT", [128, KD * H], F32, kind="ExternalInput")
    embB = nc.dram_tensor("embB", [128, KH], F32, kind="ExternalInput")
    Wab = nc.dram_tensor("Wab", [2, NB, 128, KH * H], F16, kind="ExternalInput")
    bT = nc.dram_tensor("bT", [128, NB * KH], F32, kind="ExternalInput")
    headWT = nc.dram_tensor("headWT", [128, KH * DOUT], F32, kind="ExternalInput")
    headB = nc.dram_tensor("headB", [128, KO], F32, kind="ExternalInput")
    outT = nc.dram_tensor("outT", [128, KO * BL], F32, kind="ExternalOutput")

    with TileContext(nc) as tc:
        with (
            tc.tile_pool(name="const", bufs=1) as cpool,
            tc.tile_pool(name="state", bufs=1) as spool,
            tc.tile_pool(name="wts", bufs=2) as wpool,
            tc.tile_pool(name="work", bufs=2) as kpool,
            tc.tile_pool(name="small", bufs=8) as mpool,
            tc.tile_pool(name="psum", bufs=1, space="PSUM") as ppool,
        ):
            # ---- constants ----
            xT_sb = cpool.tile([128, KD * BL], F32, tag="xt", bufs=1)
            embWT_sb = cpool.tile([128, KD * H], F32, tag="embwt", bufs=1)
            embB_sb = cpool.tile([128, KH], F32, tag="embb", bufs=1)
            bT_sb = cpool.tile([128, NB * KH], F32, tag="bt", bufs=1)
            headWT_sb = cpool.tile([128, KH * DOUT], F32, tag="hwt", bufs=1)
            headB_sb = cpool.tile([128, KO], F32, tag="hb", bufs=1)
            nc.gpsimd.dma_start(xT_sb[:], xT[:])
            nc.gpsimd.dma_start(embWT_sb[:], embWT[:])
            nc.gpsimd.dma_start(embB_sb[:], embB[:])
            nc.gpsimd.dma_start(bT_sb[:], bT[:])
            nc.gpsimd.dma_start(headWT_sb[:], headWT[:])
            nc.gpsimd.dma_start(headB_sb[:], headB[:])
            # Stage every constant through a DVE copy: downstream consumers
            # then depend on a single (DVE) semaphore. Self-loading fp32
            # matmuls only have ONE sync-wait slot in their LW struct, so
            # they cannot wait on two DMA queues directly.
            xT2 = cpool.tile([128, KD * BL], F32, tag="xt2", bufs=1)
            embWT2 = cpool.tile([128, KD * H], F32, tag="embwt2", bufs=1)
            embB2 = cpool.tile([128, KH], F32, tag="embb2", bufs=1)
            bT2 = cpool.tile([128, NB * KH], F32, tag="bt2", bufs=1)
            headWT2 = cpool.tile([128, KH * DOUT], F32, tag="hwt2", bufs=1)
            headB2 = cpool.tile([128, KO], F32, tag="hb2", bufs=1)
            for dst, srcv in ((xT2, xT_sb), (embWT2, embWT_sb), (embB2, embB_sb),
                              (bT2, bT_sb), (headWT2, headWT_sb), (headB2, headB_sb)):
                nc.vector.tensor_copy(dst[:], srcv[:])
            xT_sb, embWT_sb, embB_sb, bT_sb, headWT_sb, headB_sb = (
                xT2, embWT2, embB2, bT2, headWT2, headB2)

            # ---- persistent state (transposed layout) ----
            v = [spool.tile([128, H], F32, tag=f"v{n}", bufs=1, name=f"v{n}") for n in range(NB)]
            xemb = spool.tile([128, H], F32, tag="xemb", bufs=1)

            # per-m-tile PSUM banks (each [128,128] fp32 tile pads to one bank)
            def psm(m):
                return ppool.tile([128, 128], F32, tag=f"ps{m}", bufs=1, name=f"ps{m}")

            for n in range(NB):
                nc.vector.memset(v[n][:], 0.0)

            # ---- embed: xemb = (x @ embed_W.T + embed_b)^T ----
            for m in range(KH):
                pe = psm(m)
                for k in range(KD):
                    nc.tensor.matmul(
                        pe[:],
                        embWT_sb[:, k * H + m * 128 : k * H + (m + 1) * 128],
                        xT_sb[:, k * BL : (k + 1) * BL],
                        start=(k == 0),
                        stop=(k == KD - 1),
                    )
                nc.scalar.activation(
                    xemb[:, m * 128 : (m + 1) * 128], pe[:], Ident,
                    bias=embB_sb[:, m : m + 1], scale=1.0,
                )

            # ---- main recurrence ----
            # Blocks n and n-1 within a step are mutually independent (each
            # reads only previous-step state), so process them as a software-
            # pipelined pair: their matmul iterations interleave on the PE and
            # each block's psum->add->tanh->axpy chain hides under the other
            # block's 64-matmul group instead of stalling the PE. PSUM layout
            # (one bank per m-tile) is unchanged from the validated baseline.
            for step in range(steps):
                par = step % 2
                for pn in range(NB - 1, 0, -2):
                    pair = (pn, pn - 1)
                    wsets, cs, vhs, us = {}, {}, {}, {}
                    for n in pair:
                        w = []
                        for k in range(KH):
                            wk = wpool.tile([128, H], F16, tag=f"w{k}", bufs=2, name=f"w{k}")
                            nc.sync.dma_start(
                                wk[:], Wab[par, n, :, k * H : (k + 1) * H]
                            )
                            w.append(wk)
                        wsets[n] = w

                        binT = xemb if n == 0 else v[n - 1]
                        c = kpool.tile([128, H], F32, tag="c", bufs=3, name="c")
                        nc.vector.tensor_add(c[:], v[n][:], binT[:])
                        vh = kpool.tile([128, H], F32, tag="vh", bufs=3, name="vh")
                        nc.vector.tensor_scalar_mul(vh[:], v[n][:], 0.5)
                        cs[n], vhs[n] = c, vh

                        # u1 = tanh(c + b[n])  (inner iter 0; state is zero)
                        u = kpool.tile([128, H], F16, tag="u", bufs=4, name="u")
                        for m in range(KH):
                            nc.scalar.activation(
                                u[:, m * 128 : (m + 1) * 128],
                                c[:, m * 128 : (m + 1) * 128],
                                Tanh, bias=bT_sb[:, n * KH + m : n * KH + m + 1],
                                scale=1.0,
                            )
                        us[n] = u

                    for j in range(1, INNER):
                        last = j == INNER - 1
                        for n in pair:
                            u, c, vh, w = us[n], cs[n], vhs[n], wsets[n]
                            un = None if last else kpool.tile(
                                [128, H], F16, tag="u", bufs=4, name="un"
                            )
                            for m in range(KH):
                                mc = slice(m * 128, (m + 1) * 128)
                                ps = psm(m)
                                for k in range(KH):
                                    nc.tensor.matmul(
                                        ps[:],
                                        w[k][:, m * 128 : (m + 1) * 128],
                                        u[:, k * 128 : (k + 1) * 128],
                                        start=(k == 0),
                                        stop=(k == KH - 1),
                                    )
                                s = mpool.tile([128, 128], F32, tag="s", bufs=8)
                                nc.vector.tensor_add(s[:], ps[:], c[:, mc])
                                t = mpool.tile([128, 128], F32, tag="t", bufs=8)
                                nc.scalar.activation(
                                    t[:], s[:], Tanh,
                                    bias=bT_sb[:, n * KH + m : n * KH + m + 1],
                                    scale=1.0,
                                )
                                if not last:
                                    nc.vector.scalar_tensor_tensor(
                                        un[:, mc], u[:, mc], 0.5, t[:], MULT, ADD
                                    )
                                else:
                                    u5 = mpool.tile([128, 128], F32, tag="u5", bufs=8)
                                    nc.vector.scalar_tensor_tensor(
                                        u5[:], u[:, mc], 0.5, t[:], MULT, ADD
                                    )
                                    nc.vector.scalar_tensor_tensor(
                                        v[n][:, mc], u5[:], 0.25, vh[:, mc], MULT, ADD
                                    )
                            if not last:
                                us[n] = un

            # ---- head: out^T = head_W @ v[9]^T + head_b ----
            outsb = kpool.tile([128, KO * BL], F32, tag="outsb", bufs=1)
            for m in range(KO):
                ph = psm(m)
                for k in range(KH):
                    nc.tensor.matmul(
                        ph[:],
                        headWT_sb[:, k * DOUT + m * 128 : k * DOUT + (m + 1) * 128],
                        v[NB - 1][:, k * 128 : (k + 1) * 128],
                        start=(k == 0),
                        stop=(k == KH - 1),
                    )
                nc.scalar.activation(
                    outsb[:, m * BL : (m + 1) * BL], ph[:], Ident,
                    bias=headB_sb[:, m : m + 1], scale=1.0,
                )
            nc.sync.dma_start(outT[:], outsb[:])
    nc.compile()
    return nc


def _tile_k(a):
    """[K, M] -> [128, (K//128)*M] laid out (k_lo, k_hi, m)."""
    K, M = a.shape
    return np.ascontiguousarray(
        a.reshape(K // 128, 128, M).transpose(1, 0, 2).reshape(128, (K // 128) * M)
    )


def kernel(**inputs) -> np.ndarray:
    x = np.asarray(inputs["x"], np.float32)
    embed_W = np.asarray(inputs["embed_W"], np.float32)
    embed_b = np.asarray(inputs["embed_b"], np.float32)
    block_W = np.asarray(inputs["block_W"], np.float32)
    block_b = np.asarray(inputs["block_b"], np.float32)
    head_W = np.asarray(inputs["head_W"], np.float32)
    head_b = np.asarray(inputs["head_b"], np.float32)
    steps = int(np.asarray(inputs["steps"]))

    embWT = _tile_k(embed_W.T)
    headWT = _tile_k(head_W.T)
    Wt = block_W.transpose(0, 2, 1) * np.float32(0.5)  # [NB, K=h_in, M=d_out]
    Wa = Wt.astype(np.float16)
    Wb = (2.0 * Wt - Wa.astype(np.float32)).astype(np.float16)
    Wab = np.stack(
        [
            np.stack([_tile_k(Wa[n]) for n in range(NB)]),
            np.stack([_tile_k(Wb[n]) for n in range(NB)]),
        ]
    )  # [2, NB, 128, 8*1024] f16
    embB = np.ascontiguousarray(embed_b.reshape(KH, 128).T)
    bT = np.ascontiguousarray(
        block_b.reshape(NB, KH, 128).transpose(2, 0, 1).reshape(128, NB * KH)
    )
    headB = np.ascontiguousarray(head_b.reshape(KO, 128).T)

    in_maps = []
    for ci in range(NCORES):
        xT = _tile_k(np.ascontiguousarray(x[ci * BL : (ci + 1) * BL].T))
        in_maps.append(
            dict(xT=xT, embWT=embWT, embB=embB, Wab=Wab, bT=bT,
                 headWT=headWT, headB=headB)
        )

    nc = build_nc(steps)
    res = run_bass_kernel_spmd(nc, in_maps, core_ids=list(range(NCORES)))

    out = np.empty((B, DOUT), np.float32)
    for ci in range(NCORES):
        oT = res.results[ci]["outT"]  # [128, (do_hi=4, b=128)] = out^T tiled
        out[ci * BL : (ci + 1) * BL] = (
            oT.reshape(128, KO, BL).transpose(2, 1, 0).reshape(BL, DOUT)
        )
    return out



# revision 2
# speedup vs baseline: 1.3784x; 1.3784x over previous
"""Trainium2 Bass kernel v2 for nn_DeepRecursiveNetwork.

Same math as baseline kernel.py, restructured for engine balance:
  - bias b[n] folded into c = v[n] + block_in (+ bfull) so tanh needs no
    per-m-tile bias -> wide [128,512] PSUM banks, wide Act/DVE ops.
  - c is written into PSUM (Act/DVE prefill), matmuls accumulate on top with
    start=False: kills the per-m-tile DVE add of the baseline.
  - Pool engine (gpsimd) takes the wide SBUF adds (c, vh) - it cannot touch
    PSUM (BIR rule) but is otherwise idle.
  - t (tanh out) and u state are fp16; only v stays fp32.
  - software pipeline at the (step, pair) "unit" level: unit i+1's weight DMA
    and phaseA (c, vh, u1, first prefill) are emitted during unit i's inner
    iterations; PSUM bank sets alternate per unit so the lookahead never
    blocks the Act/DVE queues on banks still in use.
"""

import numpy as np

import concourse.bass as bass
import concourse.bacc as bacc
import concourse.mybir as mybir
from concourse.bass_utils import run_bass_kernel_spmd
from concourse.tile import TileContext

F32 = mybir.dt.float32
F16 = mybir.dt.float16

B, DIN, H, DOUT, NB = 1024, 512, 1024, 512, 10
NCORES = 8
BL = B // NCORES  # 128
KH = H // 128     # 8
KD = DIN // 128   # 4
KO = DOUT // 128  # 4
INNER = 5
Tanh = mybir.ActivationFunctionType.Tanh
Copy = mybir.ActivationFunctionType.Copy
Ident = mybir.ActivationFunctionType.Identity
MULT = mybir.AluOpType.mult
ADD = mybir.AluOpType.add


def build_nc(steps: int, with_bias: bool = False):
    nc = bacc.Bacc(None, target_bir_lowering=False)
    xT = nc.dram_tensor("xT", [128, KD * BL], F32, kind="ExternalInput")
    embWT = nc.dram_tensor("embWT", [128, KD * H], F32, kind="ExternalInput")
    embB = nc.dram_tensor("embB", [128, KH], F32, kind="ExternalInput")
    Wab = nc.dram_tensor("Wab", [2, NB, 128, KH * H], F16, kind="ExternalInput")
    bT = nc.dram_tensor("bT", [128, NB * KH], F32, kind="ExternalInput")
    headWT = nc.dram_tensor("headWT", [128, KH * DOUT], F32, kind="ExternalInput")
    headB = nc.dram_tensor("headB", [128, KO], F32, kind="ExternalInput")
    outT = nc.dram_tensor("outT", [128, KO * BL], F32, kind="ExternalOutput")

    with TileContext(nc) as tc:
        with (
            tc.tile_pool(name="const", bufs=1) as cpool,
            tc.tile_pool(name="state", bufs=1) as spool,
            tc.tile_pool(name="wts", bufs=4) as wpool,
            tc.tile_pool(name="work", bufs=2) as kpool,
            tc.tile_pool(name="psum", bufs=1, space="PSUM") as ppool,
        ):
            # ---- constants (all on the SP DMA queue: single semaphore for
            # the fp32 self-loading embed/head matmuls) ----
            xT_sb = cpool.tile([128, KD * BL], F32, tag="xt", bufs=1)
            embWT_sb = cpool.tile([128, KD * H], F32, tag="embwt", bufs=1)
            embB_sb = cpool.tile([128, KH], F32, tag="embb", bufs=1)
            bT_sb = cpool.tile([128, NB * KH], F32, tag="bt", bufs=1)
            headWT_sb = cpool.tile([128, KH * DOUT], F32, tag="hwt", bufs=1)
            headB_sb = cpool.tile([128, KO], F32, tag="hb", bufs=1)
            nc.sync.dma_start(xT_sb[:], xT[:])
            nc.sync.dma_start(embWT_sb[:], embWT[:])
            nc.sync.dma_start(embB_sb[:], embB[:])
            nc.sync.dma_start(bT_sb[:], bT[:])
            nc.sync.dma_start(headWT_sb[:], headWT[:])
            nc.sync.dma_start(headB_sb[:], headB[:])

            # ---- persistent state ----
            v = [spool.tile([128, H], F32, tag=f"v{n}", bufs=1, name=f"v{n}")
                 for n in range(NB)]
            xemb = spool.tile([128, H], F32, tag="xemb", bufs=1)
            for n in range(NB):
                nc.vector.memset(v[n][:], 0.0)

            bfull = None
            if with_bias:
                bfull = [spool.tile([128, H], F32, tag=f"bf{n}", bufs=1,
                                    name=f"bf{n}") for n in range(NB)]
                zed = spool.tile([128, 128], F32, tag="zed", bufs=1)
                nc.vector.memset(zed[:], 0.0)
                for n in range(NB):
                    for m in range(KH):
                        nc.scalar.activation(
                            bfull[n][:, m * 128:(m + 1) * 128], zed[:], Ident,
                            bias=bT_sb[:, n * KH + m:n * KH + m + 1], scale=0.0)

            # psum banks: [set][pair-position][half]; sets alternate per unit
            pbank = [[[ppool.tile([128, 512], F32, tag=f"ps{s}{p}{h}", bufs=1,
                                  name=f"ps{s}{p}{h}")
                       for h in range(2)] for p in range(2)] for s in range(2)]

            # ---- embed: xemb = (x @ embed_W.T + embed_b)^T ----
            for m in range(KH):
                pe = pbank[0][m // 4][0][:, (m % 4) * 128:(m % 4 + 1) * 128]
                for k in range(KD):
                    nc.tensor.matmul(
                        pe,
                        embWT_sb[:, k * H + m * 128:k * H + (m + 1) * 128],
                        xT_sb[:, k * BL:(k + 1) * BL],
                        start=(k == 0), stop=(k == KD - 1),
                    )
                nc.scalar.activation(
                    xemb[:, m * 128:(m + 1) * 128], pe, Ident,
                    bias=embB_sb[:, m:m + 1], scale=1.0,
                )

            # ---- main recurrence, software-pipelined units ----
            units = [(step, pn) for step in range(steps)
                     for pn in range(NB - 1, 0, -2)]
            state = {}  # per live unit: dict n -> (w, c, vh, u)

            def emit_phase_a(i):
                step, pn = units[i]
                par = step % 2
                S = i % 2
                ust = {}
                for pi, n in enumerate((pn, pn - 1)):
                    w = []
                    for k in range(KH):
                        wk = wpool.tile([128, H], F16, tag=f"w{k}", bufs=4,
                                        name=f"w{k}")
                        nc.sync.dma_start(
                            wk[:], Wab[par, n, :, k * H:(k + 1) * H])
                        w.append(wk)
                    binT = xemb if n == 0 else v[n - 1]
                    c = kpool.tile([128, H], F32, tag="c", bufs=4, name="c")
                    if with_bias:
                        cv = kpool.tile([128, H], F32, tag="cv", bufs=4,
                                        name="cv")
                        nc.gpsimd.tensor_tensor(cv[:], v[n][:], binT[:], ADD)
                        nc.gpsimd.tensor_tensor(c[:], cv[:], bfull[n][:], ADD)
                    else:
                        nc.gpsimd.tensor_tensor(c[:], v[n][:], binT[:], ADD)
                    vh = kpool.tile([128, H], F32, tag="vh", bufs=4, name="vh")
                    nc.gpsimd.tensor_scalar_mul(vh[:], v[n][:], 0.5)
                    u = kpool.tile([128, H], F16, tag="u", bufs=6, name="u")
                    nc.scalar.activation(u[:], c[:], Tanh, bias=0.0, scale=1.0)
                    # prefill j=1 banks
                    nc.scalar.activation(pbank[S][pi][0][:], c[:, 0:512],
                                         Copy, scale=1.0)
                    nc.vector.tensor_copy(pbank[S][pi][1][:], c[:, 512:1024])
                    ust[n] = [w, c, vh, u]
                state[i] = ust

            def emit_phase_b(i):
                step, pn = units[i]
                S = i % 2
                ust = state.pop(i)
                for j in range(1, INNER):
                    last = j == INNER - 1
                    for pi, n in enumerate((pn, pn - 1)):
                        w, c, vh, u = ust[n]
                        t = kpool.tile([128, H], F16, tag="t", bufs=4,
                                       name="t")
                        for h in range(2):
                            bank = pbank[S][pi][h]
                            for m in range(4):
                                mg = h * 4 + m
                                for k in range(KH):
                                    nc.tensor.matmul(
                                        bank[:, m * 128:(m + 1) * 128],
                                        w[k][:, mg * 128:(mg + 1) * 128],
                                        u[:, k * 128:(k + 1) * 128],
                                        start=False, stop=(k == KH - 1),
                                        skip_group_check=(k == 0),
                                    )
                            nc.scalar.activation(
                                t[:, h * 512:(h + 1) * 512], bank[:], Tanh,
                                bias=0.0, scale=1.0)
                            if not last:
                                # prefill for next iteration
                                if h == 0:
                                    nc.scalar.activation(
                                        bank[:], c[:, 0:512], Copy, scale=1.0)
                                else:
                                    nc.vector.tensor_copy(
                                        bank[:], c[:, 512:1024])
                        if not last:
                            un = kpool.tile([128, H], F16, tag="u", bufs=6,
                                            name="un")
                            nc.vector.scalar_tensor_tensor(
                                un[:], u[:], 0.5, t[:], MULT, ADD)
                            ust[n][3] = un
                        else:
                            u5 = kpool.tile([128, H], F16, tag="u5", bufs=2,
                                            name="u5")
                            nc.vector.scalar_tensor_tensor(
                                u5[:], u[:], 0.5, t[:], MULT, ADD)
                            nc.vector.scalar_tensor_tensor(
                                v[n][:], u5[:], 0.25, vh[:], MULT, ADD)

            emit_phase_a(0)
            for i in range(len(units)):
                if i + 1 < len(units):
                    emit_phase_a(i + 1)
                emit_phase_b(i)

            # ---- head: out^T = head_W @ v[9]^T + head_b ----
            outsb = kpool.tile([128, KO * BL], F32, tag="outsb", bufs=1)
            for m in range(KO):
                ph = pbank[0][0][0][:, m * 128:(m + 1) * 128]
                for k in range(KH):
                    nc.tensor.matmul(
                        ph,
                        headWT_sb[:, k * DOUT + m * 128:k * DOUT + (m + 1) * 128],
                        v[NB - 1][:, k * 128:(k + 1) * 128],
                        start=(k == 0), stop=(k == KH - 1),
                    )
                nc.scalar.activation(
                    outsb[:, m * BL:(m + 1) * BL], ph, Ident,
                    bias=headB_sb[:, m:m + 1], scale=1.0,
                )
            nc.sync.dma_start(outT[:], outsb[:])
    nc.compile()
    return nc


def _tile_k(a):
    """[K, M] -> [128, (K//128)*M] laid out (k_lo, k_hi, m)."""
    K, M = a.shape
    return np.ascontiguousarray(
        a.reshape(K // 128, 128, M).transpose(1, 0, 2).reshape(128, (K // 128) * M)
    )


def kernel(**inputs) -> np.ndarray:
    x = np.asarray(inputs["x"], np.float32)
    embed_W = np.asarray(inputs["embed_W"], np.float32)
    embed_b = np.asarray(inputs["embed_b"], np.float32)
    block_W = np.asarray(inputs["block_W"], np.float32)
    block_b = np.asarray(inputs["block_b"], np.float32)
    head_W = np.asarray(inputs["head_W"], np.float32)
    head_b = np.asarray(inputs["head_b"], np.float32)
    steps = int(np.asarray(inputs["steps"]))
    with_bias = bool(np.any(block_b))

    embWT = _tile_k(embed_W.T)
    headWT = _tile_k(head_W.T)
    Wt = block_W.transpose(0, 2, 1) * np.float32(0.5)  # [NB, K=h_in, M=d_out]
    Wa = Wt.astype(np.float16)
    Wb = (2.0 * Wt - Wa.astype(np.float32)).astype(np.float16)
    Wab = np.stack(
        [
            np.stack([_tile_k(Wa[n]) for n in range(NB)]),
            np.stack([_tile_k(Wb[n]) for n in range(NB)]),
        ]
    )  # [2, NB, 128, 8*1024] f16
    embB = np.ascontiguousarray(embed_b.reshape(KH, 128).T)
    bT = np.ascontiguousarray(
        block_b.reshape(NB, KH, 128).transpose(2, 0, 1).reshape(128, NB * KH)
    )
    headB = np.ascontiguousarray(head_b.reshape(KO, 128).T)

    in_maps = []
    for ci in range(NCORES):
        xTl = _tile_k(np.ascontiguousarray(x[ci * BL:(ci + 1) * BL].T))
        in_maps.append(
            dict(xT=xTl, embWT=embWT, embB=embB, Wab=Wab, bT=bT,
                 headWT=headWT, headB=headB)
        )

    nc = build_nc(steps, with_bias)
    res = run_bass_kernel_spmd(nc, in_maps, core_ids=list(range(NCORES)))

    out = np.empty((B, DOUT), np.float32)
    for ci in range(NCORES):
        oT = res.results[ci]["outT"]
        out[ci * BL:(ci + 1) * BL] = (
            oT.reshape(128, KO, BL).transpose(2, 1, 0).reshape(BL, DOUT)
        )
    return out


# revision 3
# speedup vs baseline: 1.3793x; 1.0007x over previous
"""Trainium2 Bass kernel v2 for nn_DeepRecursiveNetwork.

Same math as baseline kernel.py, restructured for engine balance:
  - bias b[n] folded into c = v[n] + block_in (+ bfull) so tanh needs no
    per-m-tile bias -> wide [128,512] PSUM banks, wide Act/DVE ops.
  - c is written into PSUM (Act/DVE prefill), matmuls accumulate on top with
    start=False: kills the per-m-tile DVE add of the baseline.
  - Pool engine (gpsimd) takes the wide SBUF adds (c, vh) - it cannot touch
    PSUM (BIR rule) but is otherwise idle.
  - t (tanh out) and u state are fp16; only v stays fp32.
  - software pipeline at the (step, pair) "unit" level: unit i+1's weight DMA
    and phaseA (c, vh, u1, first prefill) are emitted during unit i's inner
    iterations; PSUM bank sets alternate per unit so the lookahead never
    blocks the Act/DVE queues on banks still in use.
"""

import numpy as np

import concourse.bass as bass
import concourse.bacc as bacc
import concourse.mybir as mybir
from concourse.bass_utils import run_bass_kernel_spmd
from concourse.tile import TileContext

F32 = mybir.dt.float32
F16 = mybir.dt.float16

B, DIN, H, DOUT, NB = 1024, 512, 1024, 512, 10
NCORES = 8
BL = B // NCORES  # 128
KH = H // 128     # 8
KD = DIN // 128   # 4
KO = DOUT // 128  # 4
INNER = 5
Tanh = mybir.ActivationFunctionType.Tanh
Copy = mybir.ActivationFunctionType.Copy
Ident = mybir.ActivationFunctionType.Identity
MULT = mybir.AluOpType.mult
ADD = mybir.AluOpType.add


def build_nc(steps: int, with_bias: bool = False):
    nc = bacc.Bacc(None, target_bir_lowering=False)
    xT = nc.dram_tensor("xT", [128, KD * BL], F16, kind="ExternalInput")
    embWT = nc.dram_tensor("embWT", [128, KD * H], F16, kind="ExternalInput")
    embB = nc.dram_tensor("embB", [128, KH], F32, kind="ExternalInput")
    Wab = nc.dram_tensor("Wab", [2, NB, 128, KH * H], F16, kind="ExternalInput")
    bT = nc.dram_tensor("bT", [128, NB * KH], F32, kind="ExternalInput")
    headWT = nc.dram_tensor("headWT", [128, KH * DOUT], F16, kind="ExternalInput")
    headB = nc.dram_tensor("headB", [128, KO], F32, kind="ExternalInput")
    outT = nc.dram_tensor("outT", [128, KO * BL], F32, kind="ExternalOutput")

    with TileContext(nc) as tc:
        with (
            tc.tile_pool(name="const", bufs=1) as cpool,
            tc.tile_pool(name="state", bufs=1) as spool,
            tc.tile_pool(name="wts", bufs=4) as wpool,
            tc.tile_pool(name="work", bufs=2) as kpool,
            tc.tile_pool(name="psum", bufs=1, space="PSUM") as ppool,
        ):
            # ---- constants (all on the SP DMA queue: single semaphore for
            # the fp32 self-loading embed/head matmuls) ----
            xT_sb = cpool.tile([128, KD * BL], F16, tag="xt", bufs=1)
            embWT_sb = cpool.tile([128, KD * H], F16, tag="embwt", bufs=1)
            embB_sb = cpool.tile([128, KH], F32, tag="embb", bufs=1)
            bT_sb = cpool.tile([128, NB * KH], F32, tag="bt", bufs=1)
            headWT_sb = cpool.tile([128, KH * DOUT], F16, tag="hwt", bufs=1)
            headB_sb = cpool.tile([128, KO], F32, tag="hb", bufs=1)
            nc.sync.dma_start(xT_sb[:], xT[:])
            nc.sync.dma_start(embWT_sb[:], embWT[:])
            nc.sync.dma_start(embB_sb[:], embB[:])
            nc.sync.dma_start(bT_sb[:], bT[:])
            nc.sync.dma_start(headWT_sb[:], headWT[:])
            nc.sync.dma_start(headB_sb[:], headB[:])

            # ---- persistent state ----
            v = [spool.tile([128, H], F32, tag=f"v{n}", bufs=1, name=f"v{n}")
                 for n in range(NB)]
            xemb = spool.tile([128, H], F32, tag="xemb", bufs=1)
            for n in range(NB):
                nc.vector.memset(v[n][:], 0.0)

            bfull = None
            if with_bias:
                bfull = [spool.tile([128, H], F32, tag=f"bf{n}", bufs=1,
                                    name=f"bf{n}") for n in range(NB)]
                zed = spool.tile([128, 128], F32, tag="zed", bufs=1)
                nc.vector.memset(zed[:], 0.0)
                for n in range(NB):
                    for m in range(KH):
                        nc.scalar.activation(
                            bfull[n][:, m * 128:(m + 1) * 128], zed[:], Ident,
                            bias=bT_sb[:, n * KH + m:n * KH + m + 1], scale=0.0)

            # psum banks: [set][pair-position][half]; sets alternate per unit
            pbank = [[[ppool.tile([128, 512], F32, tag=f"ps{s}{p}{h}", bufs=1,
                                  name=f"ps{s}{p}{h}")
                       for h in range(2)] for p in range(2)] for s in range(2)]

            # ---- embed: xemb = (x @ embed_W.T + embed_b)^T ----
            for m in range(KH):
                pe = pbank[0][m // 4][0][:, (m % 4) * 128:(m % 4 + 1) * 128]
                for k in range(KD):
                    nc.tensor.matmul(
                        pe,
                        embWT_sb[:, k * H + m * 128:k * H + (m + 1) * 128],
                        xT_sb[:, k * BL:(k + 1) * BL],
                        start=(k == 0), stop=(k == KD - 1),
                    )
                nc.scalar.activation(
                    xemb[:, m * 128:(m + 1) * 128], pe, Ident,
                    bias=embB_sb[:, m:m + 1], scale=1.0,
                )

            # ---- main recurrence, software-pipelined units ----
            units = [(step, pn) for step in range(steps)
                     for pn in range(NB - 1, 0, -2)]
            state = {}  # per live unit: dict n -> (w, c, vh, u)

            def emit_phase_a(i):
                step, pn = units[i]
                par = step % 2
                S = i % 2
                ust = {}
                for pi, n in enumerate((pn, pn - 1)):
                    w = []
                    for k in range(KH):
                        wk = wpool.tile([128, H], F16, tag=f"w{k}", bufs=4,
                                        name=f"w{k}")
                        nc.sync.dma_start(
                            wk[:], Wab[par, n, :, k * H:(k + 1) * H])
                        w.append(wk)
                    binT = xemb if n == 0 else v[n - 1]
                    c = kpool.tile([128, H], F32, tag="c", bufs=4, name="c")
                    if with_bias:
                        cv = kpool.tile([128, H], F32, tag="cv", bufs=4,
                                        name="cv")
                        nc.gpsimd.tensor_tensor(cv[:], v[n][:], binT[:], ADD)
                        nc.gpsimd.tensor_tensor(c[:], cv[:], bfull[n][:], ADD)
                    else:
                        nc.gpsimd.tensor_tensor(c[:], v[n][:], binT[:], ADD)
                    vh = kpool.tile([128, H], F32, tag="vh", bufs=4, name="vh")
                    nc.gpsimd.tensor_scalar_mul(vh[:], v[n][:], 0.5)
                    u = kpool.tile([128, H], F16, tag="u", bufs=6, name="u")
                    nc.scalar.activation(u[:], c[:], Tanh, bias=0.0, scale=1.0)
                    # prefill j=1 banks
                    nc.scalar.activation(pbank[S][pi][0][:], c[:, 0:512],
                                         Copy, scale=1.0)
                    nc.vector.tensor_copy(pbank[S][pi][1][:], c[:, 512:1024])
                    ust[n] = [w, c, vh, u]
                state[i] = ust

            def emit_phase_b(i):
                step, pn = units[i]
                S = i % 2
                ust = state.pop(i)
                for j in range(1, INNER):
                    last = j == INNER - 1
                    for pi, n in enumerate((pn, pn - 1)):
                        w, c, vh, u = ust[n]
                        t = kpool.tile([128, H], F16, tag="t", bufs=4,
                                       name="t")
                        for h in range(2):
                            bank = pbank[S][pi][h]
                            for m in range(4):
                                mg = h * 4 + m
                                for k in range(KH):
                                    nc.tensor.matmul(
                                        bank[:, m * 128:(m + 1) * 128],
                                        w[k][:, mg * 128:(mg + 1) * 128],
                                        u[:, k * 128:(k + 1) * 128],
                                        start=False, stop=(k == KH - 1),
                                        skip_group_check=(k == 0),
                                    )
                            nc.scalar.activation(
                                t[:, h * 512:(h + 1) * 512], bank[:], Tanh,
                                bias=0.0, scale=1.0)
                            if not last:
                                # prefill for next iteration
                                if h == 0:
                                    nc.scalar.activation(
                                        bank[:], c[:, 0:512], Copy, scale=1.0)
                                else:
                                    nc.vector.tensor_copy(
                                        bank[:], c[:, 512:1024])
                        if not last:
                            un = kpool.tile([128, H], F16, tag="u", bufs=6,
                                            name="un")
                            nc.vector.scalar_tensor_tensor(
                                un[:], u[:], 0.5, t[:], MULT, ADD)
                            ust[n][3] = un
                        else:
                            u5 = kpool.tile([128, H], F16, tag="u5", bufs=2,
                                            name="u5")
                            nc.vector.scalar_tensor_tensor(
                                u5[:], u[:], 0.5, t[:], MULT, ADD)
                            nc.vector.scalar_tensor_tensor(
                                v[n][:], u5[:], 0.25, vh[:], MULT, ADD)

            emit_phase_a(0)
            for i in range(len(units)):
                if i + 1 < len(units):
                    emit_phase_a(i + 1)
                emit_phase_b(i)

            # ---- head: out^T = head_W @ v[9]^T + head_b ----
            outsb = kpool.tile([128, KO * BL], F32, tag="outsb", bufs=1)
            v9h = kpool.tile([128, H], F16, tag="v9h", bufs=1)
            nc.vector.tensor_copy(v9h[:], v[NB - 1][:])
            for m in range(KO):
                ph = pbank[0][0][0][:, m * 128:(m + 1) * 128]
                for k in range(KH):
                    nc.tensor.matmul(
                        ph,
                        headWT_sb[:, k * DOUT + m * 128:k * DOUT + (m + 1) * 128],
                        v9h[:, k * 128:(k + 1) * 128],
                        start=(k == 0), stop=(k == KH - 1),
                    )
                nc.scalar.activation(
                    outsb[:, m * BL:(m + 1) * BL], ph, Ident,
                    bias=headB_sb[:, m:m + 1], scale=1.0,
                )
            nc.sync.dma_start(outT[:], outsb[:])
    nc.compile()
    return nc


def _tile_k(a):
    """[K, M] -> [128, (K//128)*M] laid out (k_lo, k_hi, m)."""
    K, M = a.shape
    return np.ascontiguousarray(
        a.reshape(K // 128, 128, M).transpose(1, 0, 2).reshape(128, (K // 128) * M)
    )


def kernel(**inputs) -> np.ndarray:
    x = np.asarray(inputs["x"], np.float32)
    embed_W = np.asarray(inputs["embed_W"], np.float32)
    embed_b = np.asarray(inputs["embed_b"], np.float32)
    block_W = np.asarray(inputs["block_W"], np.float32)
    block_b = np.asarray(inputs["block_b"], np.float32)
    head_W = np.asarray(inputs["head_W"], np.float32)
    head_b = np.asarray(inputs["head_b"], np.float32)
    steps = int(np.asarray(inputs["steps"]))
    with_bias = bool(np.any(block_b))

    embWT = _tile_k(embed_W.T).astype(np.float16)
    headWT = _tile_k(head_W.T).astype(np.float16)
    Wt = block_W.transpose(0, 2, 1) * np.float32(0.5)  # [NB, K=h_in, M=d_out]
    Wa = Wt.astype(np.float16)
    Wb = (2.0 * Wt - Wa.astype(np.float32)).astype(np.float16)
    Wab = np.stack(
        [
            np.stack([_tile_k(Wa[n]) for n in range(NB)]),
            np.stack([_tile_k(Wb[n]) for n in range(NB)]),
        ]
    )  # [2, NB, 128, 8*1024] f16
    embB = np.ascontiguousarray(embed_b.reshape(KH, 128).T)
    bT = np.ascontiguousarray(
        block_b.reshape(NB, KH, 128).transpose(2, 0, 1).reshape(128, NB * KH)
    )
    headB = np.ascontiguousarray(head_b.reshape(KO, 128).T)

    in_maps = []
    for ci in range(NCORES):
        xTl = _tile_k(np.ascontiguousarray(x[ci * BL:(ci + 1) * BL].T)).astype(np.float16)
        in_maps.append(
            dict(xT=xTl, embWT=embWT, embB=embB, Wab=Wab, bT=bT,
                 headWT=headWT, headB=headB)
        )

    nc = build_nc(steps, with_bias)
    res = run_bass_kernel_spmd(nc, in_maps, core_ids=list(range(NCORES)))

    out = np.empty((B, DOUT), np.float32)
    for ci in range(NCORES):
        oT = res.results[ci]["outT"]
        out[ci * BL:(ci + 1) * BL] = (
            oT.reshape(128, KO, BL).transpose(2, 1, 0).reshape(BL, DOUT)
        )
    return out


# revision 7
# speedup vs baseline: 1.3802x; 1.0006x over previous
"""Trainium2 Bass kernel v2 for nn_DeepRecursiveNetwork.

Same math as baseline kernel.py, restructured for engine balance:
  - bias b[n] folded into c = v[n] + block_in (+ bfull) so tanh needs no
    per-m-tile bias -> wide [128,512] PSUM banks, wide Act/DVE ops.
  - c is written into PSUM (Act/DVE prefill), matmuls accumulate on top with
    start=False: kills the per-m-tile DVE add of the baseline.
  - Pool engine (gpsimd) takes the wide SBUF adds (c, vh) - it cannot touch
    PSUM (BIR rule) but is otherwise idle.
  - t (tanh out) and u state are fp16; only v stays fp32.
  - software pipeline at the (step, pair) "unit" level: unit i+1's weight DMA
    and phaseA (c, vh, u1, first prefill) are emitted during unit i's inner
    iterations; PSUM bank sets alternate per unit so the lookahead never
    blocks the Act/DVE queues on banks still in use.
"""

import numpy as np

import concourse.bass as bass
import concourse.bacc as bacc
import concourse.mybir as mybir
from concourse.bass_utils import run_bass_kernel_spmd
from concourse.tile import TileContext

F32 = mybir.dt.float32
F16 = mybir.dt.float16

B, DIN, H, DOUT, NB = 1024, 512, 1024, 512, 10
NCORES = 8
BL = B // NCORES  # 128
KH = H // 128     # 8
KD = DIN // 128   # 4
KO = DOUT // 128  # 4
INNER = 5
Tanh = mybir.ActivationFunctionType.Tanh
Copy = mybir.ActivationFunctionType.Copy
Ident = mybir.ActivationFunctionType.Identity
MULT = mybir.AluOpType.mult
ADD = mybir.AluOpType.add


def build_nc(steps: int, with_bias: bool = False):
    nc = bacc.Bacc(None, target_bir_lowering=False)
    xT = nc.dram_tensor("xT", [128, KD * BL], F16, kind="ExternalInput")
    embWT = nc.dram_tensor("embWT", [128, KD * H], F16, kind="ExternalInput")
    embB = nc.dram_tensor("embB", [128, KH], F32, kind="ExternalInput")
    Wab = nc.dram_tensor("Wab", [2, NB, 128, KH * H], F16, kind="ExternalInput")
    bT = nc.dram_tensor("bT", [128, NB * KH], F32, kind="ExternalInput")
    headWT = nc.dram_tensor("headWT", [128, KH * DOUT], F16, kind="ExternalInput")
    headB = nc.dram_tensor("headB", [128, KO], F32, kind="ExternalInput")
    outT = nc.dram_tensor("outT", [128, KO * BL], F32, kind="ExternalOutput")

    with TileContext(nc) as tc:
        with (
            tc.tile_pool(name="const", bufs=1) as cpool,
            tc.tile_pool(name="state", bufs=1) as spool,
            tc.tile_pool(name="wts", bufs=4) as wpool,
            tc.tile_pool(name="work", bufs=2) as kpool,
            tc.tile_pool(name="psum", bufs=1, space="PSUM") as ppool,
        ):
            # ---- constants (all on the SP DMA queue: single semaphore for
            # the fp32 self-loading embed/head matmuls) ----
            xT_sb = cpool.tile([128, KD * BL], F16, tag="xt", bufs=1)
            embWT_sb = cpool.tile([128, KD * H], F16, tag="embwt", bufs=1)
            embB_sb = cpool.tile([128, KH], F32, tag="embb", bufs=1)
            bT_sb = cpool.tile([128, NB * KH], F32, tag="bt", bufs=1)
            headWT_sb = cpool.tile([128, KH * DOUT], F16, tag="hwt", bufs=1)
            headB_sb = cpool.tile([128, KO], F32, tag="hb", bufs=1)
            nc.sync.dma_start(xT_sb[:], xT[:])
            nc.sync.dma_start(embWT_sb[:], embWT[:])
            nc.sync.dma_start(embB_sb[:], embB[:])
            if with_bias:
                nc.sync.dma_start(bT_sb[:], bT[:])

            # ---- persistent state ----
            v = [spool.tile([128, H], F32, tag=f"v{n}", bufs=1, name=f"v{n}")
                 for n in range(NB)]
            xemb = spool.tile([128, H], F32, tag="xemb", bufs=1)
            for n in range(NB):
                nc.vector.memset(v[n][:], 0.0)

            bfull = None
            if with_bias:
                bfull = [spool.tile([128, H], F32, tag=f"bf{n}", bufs=1,
                                    name=f"bf{n}") for n in range(NB)]
                zed = spool.tile([128, 128], F32, tag="zed", bufs=1)
                nc.vector.memset(zed[:], 0.0)
                for n in range(NB):
                    for m in range(KH):
                        nc.scalar.activation(
                            bfull[n][:, m * 128:(m + 1) * 128], zed[:], Ident,
                            bias=bT_sb[:, n * KH + m:n * KH + m + 1], scale=0.0)

            # psum banks: [set][pair-position][half]; sets alternate per unit
            pbank = [[[ppool.tile([128, 512], F32, tag=f"ps{s}{p}{h}", bufs=1,
                                  name=f"ps{s}{p}{h}")
                       for h in range(2)] for p in range(2)] for s in range(2)]

            def emit_embed():
                for m in range(KH):
                    pe = pbank[1][m // 4][0][:, (m % 4) * 128:(m % 4 + 1) * 128]
                    for k in range(KD):
                        nc.tensor.matmul(
                            pe,
                            embWT_sb[:, k * H + m * 128:k * H + (m + 1) * 128],
                            xT_sb[:, k * BL:(k + 1) * BL],
                            start=(k == 0), stop=(k == KD - 1),
                        )
                    nc.scalar.activation(
                        xemb[:, m * 128:(m + 1) * 128], pe, Ident,
                        bias=embB_sb[:, m:m + 1], scale=1.0,
                    )

            # ---- main recurrence, software-pipelined units ----
            units = [(step, pn) for step in range(steps)
                     for pn in range(NB - 1, 0, -2)]
            state = {}  # per live unit: dict n -> (w, c, vh, u)

            def emit_phase_a(i):
                step, pn = units[i]
                par = step % 2
                S = i % 2
                ust = {}
                for pi, n in enumerate((pn, pn - 1)):
                    w = []
                    for k in range(KH):
                        wk = wpool.tile([128, H], F16, tag=f"w{k}", bufs=4,
                                        name=f"w{k}")
                        nc.sync.dma_start(
                            wk[:], Wab[par, n, :, k * H:(k + 1) * H])
                        w.append(wk)
                    binT = xemb if n == 0 else v[n - 1]
                    c = kpool.tile([128, H], F32, tag="c", bufs=4, name="c")
                    if with_bias:
                        cv = kpool.tile([128, H], F32, tag="cv", bufs=4,
                                        name="cv")
                        nc.gpsimd.tensor_tensor(cv[:], v[n][:], binT[:], ADD)
                        nc.gpsimd.tensor_tensor(c[:], cv[:], bfull[n][:], ADD)
                    else:
                        nc.gpsimd.tensor_tensor(c[:], v[n][:], binT[:], ADD)
                    vh = kpool.tile([128, H], F32, tag="vh", bufs=4, name="vh")
                    nc.gpsimd.tensor_scalar_mul(vh[:], v[n][:], 0.5)
                    u = kpool.tile([128, H], F16, tag="u", bufs=6, name="u")
                    nc.scalar.activation(u[:], c[:], Tanh, bias=0.0, scale=1.0)
                    # prefill j=1 banks
                    nc.scalar.activation(pbank[S][pi][0][:], c[:, 0:512],
                                         Copy, scale=1.0)
                    nc.vector.tensor_copy(pbank[S][pi][1][:], c[:, 512:1024])
                    ust[n] = [w, c, vh, u]
                state[i] = ust

            def emit_phase_b(i):
                step, pn = units[i]
                S = i % 2
                ust = state.pop(i)
                for j in range(1, INNER):
                    last = j == INNER - 1
                    for pi, n in enumerate((pn, pn - 1)):
                        w, c, vh, u = ust[n]
                        t = kpool.tile([128, H], F16, tag="t", bufs=4,
                                       name="t")
                        for h in range(2):
                            bank = pbank[S][pi][h]
                            for m in range(4):
                                mg = h * 4 + m
                                for k in range(KH):
                                    nc.tensor.matmul(
                                        bank[:, m * 128:(m + 1) * 128],
                                        w[k][:, mg * 128:(mg + 1) * 128],
                                        u[:, k * 128:(k + 1) * 128],
                                        start=False, stop=(k == KH - 1),
                                        skip_group_check=(k == 0),
                                    )
                            nc.scalar.activation(
                                t[:, h * 512:(h + 1) * 512], bank[:], Tanh,
                                bias=0.0, scale=1.0)
                            if not last:
                                # prefill for next iteration
                                if h == 0:
                                    nc.scalar.activation(
                                        bank[:], c[:, 0:512], Copy, scale=1.0)
                                else:
                                    nc.vector.tensor_copy(
                                        bank[:], c[:, 512:1024])
                        if not last:
                            un = kpool.tile([128, H], F16, tag="u", bufs=6,
                                            name="un")
                            nc.vector.scalar_tensor_tensor(
                                un[:], u[:], 0.5, t[:], MULT, ADD)
                            ust[n][3] = un
                        else:
                            u5 = kpool.tile([128, H], F16, tag="u5", bufs=2,
                                            name="u5")
                            nc.vector.scalar_tensor_tensor(
                                u5[:], u[:], 0.5, t[:], MULT, ADD)
                            nc.vector.scalar_tensor_tensor(
                                v[n][:], u5[:], 0.25, vh[:], MULT, ADD)

            emit_phase_a(0)
            emit_embed()
            for i in range(len(units)):
                if i + 1 < len(units):
                    emit_phase_a(i + 1)
                emit_phase_b(i)

            # ---- head: out^T = head_W @ v[9]^T + head_b ----
            nc.sync.dma_start(headWT_sb[:], headWT[:])
            nc.sync.dma_start(headB_sb[:], headB[:])
            outsb = kpool.tile([128, KO * BL], F32, tag="outsb", bufs=1)
            v9h = kpool.tile([128, H], F16, tag="v9h", bufs=1)
            nc.vector.tensor_copy(v9h[:], v[NB - 1][:])
            for m in range(KO):
                ph = pbank[0][0][0][:, m * 128:(m + 1) * 128]
                for k in range(KH):
                    nc.tensor.matmul(
                        ph,
                        headWT_sb[:, k * DOUT + m * 128:k * DOUT + (m + 1) * 128],
                        v9h[:, k * 128:(k + 1) * 128],
                        start=(k == 0), stop=(k == KH - 1),
                    )
                nc.scalar.activation(
                    outsb[:, m * BL:(m + 1) * BL], ph, Ident,
                    bias=headB_sb[:, m:m + 1], scale=1.0,
                )
            nc.sync.dma_start(outT[:], outsb[:])
    nc.compile()
    return nc


def _tile_k(a):
    """[K, M] -> [128, (K//128)*M] laid out (k_lo, k_hi, m)."""
    K, M = a.shape
    return np.ascontiguousarray(
        a.reshape(K // 128, 128, M).transpose(1, 0, 2).reshape(128, (K // 128) * M)
    )


def kernel(**inputs) -> np.ndarray:
    x = np.asarray(inputs["x"], np.float32)
    embed_W = np.asarray(inputs["embed_W"], np.float32)
    embed_b = np.asarray(inputs["embed_b"], np.float32)
    block_W = np.asarray(inputs["block_W"], np.float32)
    block_b = np.asarray(inputs["block_b"], np.float32)
    head_W = np.asarray(inputs["head_W"], np.float32)
    head_b = np.asarray(inputs["head_b"], np.float32)
    steps = int(np.asarray(inputs["steps"]))
    with_bias = bool(np.any(block_b))

    embWT = _tile_k(embed_W.T).astype(np.float16)
    headWT = _tile_k(head_W.T).astype(np.float16)
    Wt = block_W.transpose(0, 2, 1) * np.float32(0.5)  # [NB, K=h_in, M=d_out]
    Wa = Wt.astype(np.float16)
    Wb = (2.0 * Wt - Wa.astype(np.float32)).astype(np.float16)
    Wab = np.stack(
        [
            np.stack([_tile_k(Wa[n]) for n in range(NB)]),
            np.stack([_tile_k(Wb[n]) for n in range(NB)]),
        ]
    )  # [2, NB, 128, 8*1024] f16
    embB = np.ascontiguousarray(embed_b.reshape(KH, 128).T)
    bT = np.ascontiguousarray(
        block_b.reshape(NB, KH, 128).transpose(2, 0, 1).reshape(128, NB * KH)
    )
    headB = np.ascontiguousarray(head_b.reshape(KO, 128).T)

    in_maps = []
    for ci in range(NCORES):
        xTl = _tile_k(np.ascontiguousarray(x[ci * BL:(ci + 1) * BL].T)).astype(np.float16)
        in_maps.append(
            dict(xT=xTl, embWT=embWT, embB=embB, Wab=Wab, bT=bT,
                 headWT=headWT, headB=headB)
        )

    nc = build_nc(steps, with_bias)
    res = run_bass_kernel_spmd(nc, in_maps, core_ids=list(range(NCORES)))

    out = np.empty((B, DOUT), np.float32)
    for ci in range(NCORES):
        oT = res.results[ci]["outT"]
        out[ci * BL:(ci + 1) * BL] = (
            oT.reshape(128, KO, BL).transpose(2, 1, 0).reshape(BL, DOUT)
        )
    return out


# revision 10
# speedup vs baseline: 1.3823x; 1.0015x over previous
"""Trainium2 Bass kernel v2 for nn_DeepRecursiveNetwork.

Same math as baseline kernel.py, restructured for engine balance:
  - bias b[n] folded into c = v[n] + block_in (+ bfull) so tanh needs no
    per-m-tile bias -> wide [128,512] PSUM banks, wide Act/DVE ops.
  - c is written into PSUM (Act/DVE prefill), matmuls accumulate on top with
    start=False: kills the per-m-tile DVE add of the baseline.
  - Pool engine (gpsimd) takes the wide SBUF adds (c, vh) - it cannot touch
    PSUM (BIR rule) but is otherwise idle.
  - t (tanh out) and u state are fp16; only v stays fp32.
  - software pipeline at the (step, pair) "unit" level: unit i+1's weight DMA
    and phaseA (c, vh, u1, first prefill) are emitted during unit i's inner
    iterations; PSUM bank sets alternate per unit so the lookahead never
    blocks the Act/DVE queues on banks still in use.
"""

import numpy as np

import concourse.bass as bass
import concourse.bacc as bacc
import concourse.mybir as mybir
from concourse.bass_utils import run_bass_kernel_spmd
from concourse.tile import TileContext

F32 = mybir.dt.float32
F16 = mybir.dt.float16

B, DIN, H, DOUT, NB = 1024, 512, 1024, 512, 10
NCORES = 8
BL = B // NCORES  # 128
KH = H // 128     # 8
KD = DIN // 128   # 4
KO = DOUT // 128  # 4
INNER = 5
Tanh = mybir.ActivationFunctionType.Tanh
Copy = mybir.ActivationFunctionType.Copy
Ident = mybir.ActivationFunctionType.Identity
MULT = mybir.AluOpType.mult
ADD = mybir.AluOpType.add


def build_nc(steps: int, with_bias: bool = False):
    nc = bacc.Bacc(None, target_bir_lowering=False)
    xT = nc.dram_tensor("xT", [128, KD * BL], F16, kind="ExternalInput")
    embWT = nc.dram_tensor("embWT", [128, KD * H], F16, kind="ExternalInput")
    embB = nc.dram_tensor("embB", [128, KH], F32, kind="ExternalInput")
    Wab = nc.dram_tensor("Wab", [2, NB, 128, KH * H], F16, kind="ExternalInput")
    bT = nc.dram_tensor("bT", [128, NB * KH], F32, kind="ExternalInput")
    headWT = nc.dram_tensor("headWT", [128, KH * DOUT], F16, kind="ExternalInput")
    headB = nc.dram_tensor("headB", [128, KO], F32, kind="ExternalInput")
    outT = nc.dram_tensor("outT", [128, KO * BL], F32, kind="ExternalOutput")

    with TileContext(nc) as tc:
        with (
            tc.tile_pool(name="const", bufs=1) as cpool,
            tc.tile_pool(name="state", bufs=1) as spool,
            tc.tile_pool(name="wts", bufs=4) as wpool,
            tc.tile_pool(name="work", bufs=2) as kpool,
            tc.tile_pool(name="psum", bufs=1, space="PSUM") as ppool,
        ):
            # ---- constants (all on the SP DMA queue: single semaphore for
            # the fp32 self-loading embed/head matmuls) ----
            xT_sb = cpool.tile([128, KD * BL], F16, tag="xt", bufs=1)
            embWT_sb = cpool.tile([128, KD * H], F16, tag="embwt", bufs=1)
            embB_sb = cpool.tile([128, KH], F32, tag="embb", bufs=1)
            bT_sb = cpool.tile([128, NB * KH], F32, tag="bt", bufs=1)
            headWT_sb = cpool.tile([128, KH * DOUT], F16, tag="hwt", bufs=1)
            headB_sb = cpool.tile([128, KO], F32, tag="hb", bufs=1)
            if with_bias:
                nc.sync.dma_start(bT_sb[:], bT[:])

            # ---- persistent state ----
            v = [spool.tile([128, H], F32, tag=f"v{n}", bufs=1, name=f"v{n}")
                 for n in range(NB)]
            xemb = spool.tile([128, H], F32, tag="xemb", bufs=1)
            for n in (NB - 1, NB - 2, NB - 3):
                nc.vector.memset(v[n][:], 0.0)

            bfull = None
            if with_bias:
                bfull = [spool.tile([128, H], F32, tag=f"bf{n}", bufs=1,
                                    name=f"bf{n}") for n in range(NB)]
                zed = spool.tile([128, 128], F32, tag="zed", bufs=1)
                nc.vector.memset(zed[:], 0.0)
                for n in range(NB):
                    for m in range(KH):
                        nc.scalar.activation(
                            bfull[n][:, m * 128:(m + 1) * 128], zed[:], Ident,
                            bias=bT_sb[:, n * KH + m:n * KH + m + 1], scale=0.0)

            # psum banks: [set][pair-position][half]; sets alternate per unit
            pbank = [[[ppool.tile([128, 512], F32, tag=f"ps{s}{p}{h}", bufs=1,
                                  name=f"ps{s}{p}{h}")
                       for h in range(2)] for p in range(2)] for s in range(2)]

            def emit_embed():
                for m in range(KH):
                    pe = pbank[0][m // 4][0][:, (m % 4) * 128:(m % 4 + 1) * 128]
                    for k in range(KD):
                        nc.tensor.matmul(
                            pe,
                            embWT_sb[:, k * H + m * 128:k * H + (m + 1) * 128],
                            xT_sb[:, k * BL:(k + 1) * BL],
                            start=(k == 0), stop=(k == KD - 1),
                        )
                    nc.scalar.activation(
                        xemb[:, m * 128:(m + 1) * 128], pe, Ident,
                        bias=embB_sb[:, m:m + 1], scale=1.0,
                    )

            # ---- main recurrence, software-pipelined units ----
            units = [(step, pn) for step in range(steps)
                     for pn in range(NB - 1, 0, -2)]
            state = {}  # per live unit: dict n -> (w, c, vh, u)

            def emit_phase_a(i):
                step, pn = units[i]
                par = step % 2
                S = i % 2
                ust = {}
                for pi, n in enumerate((pn, pn - 1)):
                    w = []
                    for k in range(KH):
                        wk = wpool.tile([128, H], F16, tag=f"w{k}", bufs=4,
                                        name=f"w{k}")
                        nc.sync.dma_start(
                            wk[:], Wab[par, n, :, k * H:(k + 1) * H])
                        w.append(wk)
                    binT = xemb if n == 0 else v[n - 1]
                    c = kpool.tile([128, H], F32, tag="c", bufs=4, name="c")
                    if with_bias:
                        cv = kpool.tile([128, H], F32, tag="cv", bufs=4,
                                        name="cv")
                        nc.gpsimd.tensor_tensor(cv[:], v[n][:], binT[:], ADD)
                        nc.gpsimd.tensor_tensor(c[:], cv[:], bfull[n][:], ADD)
                    else:
                        nc.gpsimd.tensor_tensor(c[:], v[n][:], binT[:], ADD)
                    vh = kpool.tile([128, H], F32, tag="vh", bufs=4, name="vh")
                    nc.gpsimd.tensor_scalar_mul(vh[:], v[n][:], 0.5)
                    u = kpool.tile([128, H], F16, tag="u", bufs=6, name="u")
                    nc.scalar.activation(u[:], c[:], Tanh, bias=0.0, scale=1.0)
                    # prefill j=1 banks
                    nc.scalar.activation(pbank[S][pi][0][:], c[:, 0:512],
                                         Copy, scale=1.0)
                    nc.vector.tensor_copy(pbank[S][pi][1][:], c[:, 512:1024])
                    ust[n] = [w, c, vh, u]
                state[i] = ust

            def emit_phase_b(i):
                step, pn = units[i]
                S = i % 2
                ust = state.pop(i)
                for j in range(1, INNER):
                    last = j == INNER - 1
                    for pi, n in enumerate((pn, pn - 1)):
                        w, c, vh, u = ust[n]
                        t = kpool.tile([128, H], F16, tag="t", bufs=4,
                                       name="t")
                        for h in range(2):
                            bank = pbank[S][pi][h]
                            for m in range(4):
                                mg = h * 4 + m
                                for k in range(KH):
                                    nc.tensor.matmul(
                                        bank[:, m * 128:(m + 1) * 128],
                                        w[k][:, mg * 128:(mg + 1) * 128],
                                        u[:, k * 128:(k + 1) * 128],
                                        start=False, stop=(k == KH - 1),
                                        skip_group_check=(k == 0),
                                    )
                            nc.scalar.activation(
                                t[:, h * 512:(h + 1) * 512], bank[:], Tanh,
                                bias=0.0, scale=1.0)
                            if not last:
                                # prefill for next iteration
                                if h == 0:
                                    nc.scalar.activation(
                                        bank[:], c[:, 0:512], Copy, scale=1.0)
                                else:
                                    nc.vector.tensor_copy(
                                        bank[:], c[:, 512:1024])
                        if not last:
                            un = kpool.tile([128, H], F16, tag="u", bufs=6,
                                            name="un")
                            nc.vector.scalar_tensor_tensor(
                                un[:], u[:], 0.5, t[:], MULT, ADD)
                            ust[n][3] = un
                        else:
                            u5 = kpool.tile([128, H], F16, tag="u5", bufs=2,
                                            name="u5")
                            nc.vector.scalar_tensor_tensor(
                                u5[:], u[:], 0.5, t[:], MULT, ADD)
                            nc.vector.scalar_tensor_tensor(
                                v[n][:], u5[:], 0.25, vh[:], MULT, ADD)

            emit_phase_a(0)
            for n in range(NB - 4, -1, -1):
                nc.vector.memset(v[n][:], 0.0)
            nc.sync.dma_start(xT_sb[:], xT[:])
            nc.sync.dma_start(embWT_sb[:], embWT[:])
            nc.sync.dma_start(embB_sb[:], embB[:])
            for i in range(len(units)):
                if i + 1 < len(units):
                    emit_phase_a(i + 1)
                emit_phase_b(i)
                if i == 0:
                    emit_embed()

            # ---- head: out^T = head_W @ v[9]^T + head_b ----
            nc.sync.dma_start(headWT_sb[:], headWT[:])
            nc.sync.dma_start(headB_sb[:], headB[:])
            outsb = kpool.tile([128, KO * BL], F32, tag="outsb", bufs=1)
            v9h = kpool.tile([128, H], F16, tag="v9h", bufs=1)
            nc.vector.tensor_copy(v9h[:], v[NB - 1][:])
            for m in range(KO):
                ph = pbank[0][0][0][:, m * 128:(m + 1) * 128]
                for k in range(KH):
                    nc.tensor.matmul(
                        ph,
                        headWT_sb[:, k * DOUT + m * 128:k * DOUT + (m + 1) * 128],
                        v9h[:, k * 128:(k + 1) * 128],
                        start=(k == 0), stop=(k == KH - 1),
                    )
                nc.scalar.activation(
                    outsb[:, m * BL:(m + 1) * BL], ph, Ident,
                    bias=headB_sb[:, m:m + 1], scale=1.0,
                )
            nc.sync.dma_start(outT[:], outsb[:])
    nc.compile()
    return nc


def _tile_k(a):
    """[K, M] -> [128, (K//128)*M] laid out (k_lo, k_hi, m)."""
    K, M = a.shape
    return np.ascontiguousarray(
        a.reshape(K // 128, 128, M).transpose(1, 0, 2).reshape(128, (K // 128) * M)
    )


def kernel(**inputs) -> np.ndarray:
    x = np.asarray(inputs["x"], np.float32)
    embed_W = np.asarray(inputs["embed_W"], np.float32)
    embed_b = np.asarray(inputs["embed_b"], np.float32)
    block_W = np.asarray(inputs["block_W"], np.float32)
    block_b = np.asarray(inputs["block_b"], np.float32)
    head_W = np.asarray(inputs["head_W"], np.float32)
    head_b = np.asarray(inputs["head_b"], np.float32)
    steps = int(np.asarray(inputs["steps"]))
    with_bias = bool(np.any(block_b))

    embWT = _tile_k(embed_W.T).astype(np.float16)
    headWT = _tile_k(head_W.T).astype(np.float16)
    Wt = block_W.transpose(0, 2, 1) * np.float32(0.5)  # [NB, K=h_in, M=d_out]
    Wa = Wt.astype(np.float16)
    Wb = (2.0 * Wt - Wa.astype(np.float32)).astype(np.float16)
    Wab = np.stack(
        [
            np.stack([_tile_k(Wa[n]) for n in range(NB)]),
            np.stack([_tile_k(Wb[n]) for n in range(NB)]),
        ]
    )  # [2, NB, 128, 8*1024] f16
    embB = np.ascontiguousarray(embed_b.reshape(KH, 128).T)
    bT = np.ascontiguousarray(
        block_b.reshape(NB, KH, 128).transpose(2, 0, 1).reshape(128, NB * KH)
    )
    headB = np.ascontiguousarray(head_b.reshape(KO, 128).T)

    in_maps = []
    for ci in range(NCORES):
        xTl = _tile_k(np.ascontiguousarray(x[ci * BL:(ci + 1) * BL].T)).astype(np.float16)
        in_maps.append(
            dict(xT=xTl, embWT=embWT, embB=embB, Wab=Wab, bT=bT,
                 headWT=headWT, headB=headB)
        )

    nc = build_nc(steps, with_bias)
    res = run_bass_kernel_spmd(nc, in_maps, core_ids=list(range(NCORES)))

    out = np.empty((B, DOUT), np.float32)
    for ci in range(NCORES):
        oT = res.results[ci]["outT"]
        out[ci * BL:(ci + 1) * BL] = (
            oT.reshape(128, KO, BL).transpose(2, 1, 0).reshape(BL, DOUT)
        )
    return out


# revision 11
# speedup vs baseline: 1.3829x; 1.0004x over previous
"""Trainium2 Bass kernel v2 for nn_DeepRecursiveNetwork.

Same math as baseline kernel.py, restructured for engine balance:
  - bias b[n] folded into c = v[n] + block_in (+ bfull) so tanh needs no
    per-m-tile bias -> wide [128,512] PSUM banks, wide Act/DVE ops.
  - c is written into PSUM (Act/DVE prefill), matmuls accumulate on top with
    start=False: kills the per-m-tile DVE add of the baseline.
  - Pool engine (gpsimd) takes the wide SBUF adds (c, vh) - it cannot touch
    PSUM (BIR rule) but is otherwise idle.
  - t (tanh out) and u state are fp16; only v stays fp32.
  - software pipeline at the (step, pair) "unit" level: unit i+1's weight DMA
    and phaseA (c, vh, u1, first prefill) are emitted during unit i's inner
    iterations; PSUM bank sets alternate per unit so the lookahead never
    blocks the Act/DVE queues on banks still in use.
"""

import numpy as np

import concourse.bass as bass
import concourse.bacc as bacc
import concourse.mybir as mybir
from concourse.bass_utils import run_bass_kernel_spmd
from concourse.tile import TileContext

F32 = mybir.dt.float32
F16 = mybir.dt.float16

B, DIN, H, DOUT, NB = 1024, 512, 1024, 512, 10
NCORES = 8
BL = B // NCORES  # 128
KH = H // 128     # 8
KD = DIN // 128   # 4
KO = DOUT // 128  # 4
INNER = 5
Tanh = mybir.ActivationFunctionType.Tanh
Copy = mybir.ActivationFunctionType.Copy
Ident = mybir.ActivationFunctionType.Identity
MULT = mybir.AluOpType.mult
ADD = mybir.AluOpType.add


def build_nc(steps: int, with_bias: bool = False):
    nc = bacc.Bacc(None, target_bir_lowering=False)
    xT = nc.dram_tensor("xT", [128, KD * BL], F16, kind="ExternalInput")
    embWT = nc.dram_tensor("embWT", [128, KD * H], F16, kind="ExternalInput")
    embB = nc.dram_tensor("embB", [128, KH], F32, kind="ExternalInput")
    Wab = nc.dram_tensor("Wab", [2, NB, 128, KH * H], F16, kind="ExternalInput")
    bT = nc.dram_tensor("bT", [128, NB * KH], F32, kind="ExternalInput")
    headWT = nc.dram_tensor("headWT", [128, KH * DOUT], F16, kind="ExternalInput")
    headB = nc.dram_tensor("headB", [128, KO], F32, kind="ExternalInput")
    outT = nc.dram_tensor("outT", [128, KO * BL], F32, kind="ExternalOutput")

    with TileContext(nc) as tc:
        with (
            tc.tile_pool(name="const", bufs=1) as cpool,
            tc.tile_pool(name="state", bufs=1) as spool,
            tc.tile_pool(name="wts", bufs=4) as wpool,
            tc.tile_pool(name="work", bufs=2) as kpool,
            tc.tile_pool(name="psum", bufs=1, space="PSUM") as ppool,
        ):
            # ---- constants (all on the SP DMA queue: single semaphore for
            # the fp32 self-loading embed/head matmuls) ----
            xT_sb = cpool.tile([128, KD * BL], F16, tag="xt", bufs=1)
            embWT_sb = cpool.tile([128, KD * H], F16, tag="embwt", bufs=1)
            embB_sb = cpool.tile([128, KH], F32, tag="embb", bufs=1)
            bT_sb = cpool.tile([128, NB * KH], F32, tag="bt", bufs=1)
            headWT_sb = cpool.tile([128, KH * DOUT], F16, tag="hwt", bufs=1)
            headB_sb = cpool.tile([128, KO], F32, tag="hb", bufs=1)
            if with_bias:
                nc.sync.dma_start(bT_sb[:], bT[:])

            # ---- persistent state ----
            v = [spool.tile([128, H], F32, tag=f"v{n}", bufs=1, name=f"v{n}")
                 for n in range(NB)]
            xemb = spool.tile([128, H], F32, tag="xemb", bufs=1)
            for n in (NB - 1, NB - 2, NB - 3):
                nc.vector.memset(v[n][:], 0.0)

            bfull = None
            if with_bias:
                bfull = [spool.tile([128, H], F32, tag=f"bf{n}", bufs=1,
                                    name=f"bf{n}") for n in range(NB)]
                zed = spool.tile([128, 128], F32, tag="zed", bufs=1)
                nc.vector.memset(zed[:], 0.0)
                for n in range(NB):
                    for m in range(KH):
                        nc.scalar.activation(
                            bfull[n][:, m * 128:(m + 1) * 128], zed[:], Ident,
                            bias=bT_sb[:, n * KH + m:n * KH + m + 1], scale=0.0)

            # psum banks: [set][pair-position][half]; sets alternate per unit
            pbank = [[[ppool.tile([128, 512], F32, tag=f"ps{s}{p}{h}", bufs=1,
                                  name=f"ps{s}{p}{h}")
                       for h in range(2)] for p in range(2)] for s in range(2)]

            def emit_embed():
                for m in range(KH):
                    pe = pbank[0][m // 4][0][:, (m % 4) * 128:(m % 4 + 1) * 128]
                    for k in range(KD):
                        nc.tensor.matmul(
                            pe,
                            embWT_sb[:, k * H + m * 128:k * H + (m + 1) * 128],
                            xT_sb[:, k * BL:(k + 1) * BL],
                            start=(k == 0), stop=(k == KD - 1),
                        )
                    nc.scalar.activation(
                        xemb[:, m * 128:(m + 1) * 128], pe, Ident,
                        bias=embB_sb[:, m:m + 1], scale=1.0,
                    )

            # ---- main recurrence, software-pipelined units ----
            units = [(step, pn) for step in range(steps)
                     for pn in range(NB - 1, 0, -2)]
            state = {}  # per live unit: dict n -> (w, c, vh, u)

            def emit_phase_a(i):
                step, pn = units[i]
                par = step % 2
                S = i % 2
                ust = {}
                for pi, n in enumerate((pn, pn - 1)):
                    w = []
                    for k in range(KH):
                        wk = wpool.tile([128, H], F16, tag=f"w{k}", bufs=4,
                                        name=f"w{k}")
                        nc.sync.dma_start(
                            wk[:], Wab[par, n, :, k * H:(k + 1) * H])
                        w.append(wk)
                    binT = xemb if n == 0 else v[n - 1]
                    c = kpool.tile([128, H], F32, tag="c", bufs=4, name="c")
                    if with_bias:
                        cv = kpool.tile([128, H], F32, tag="cv", bufs=4,
                                        name="cv")
                        nc.gpsimd.tensor_tensor(cv[:], v[n][:], binT[:], ADD)
                        nc.gpsimd.tensor_tensor(c[:], cv[:], bfull[n][:], ADD)
                    else:
                        nc.gpsimd.tensor_tensor(c[:], v[n][:], binT[:], ADD)
                    vh = kpool.tile([128, H], F32, tag="vh", bufs=4, name="vh")
                    nc.gpsimd.tensor_scalar_mul(vh[:], v[n][:], 0.5)
                    u = kpool.tile([128, H], F16, tag="u", bufs=6, name="u")
                    nc.scalar.activation(u[:], c[:], Tanh, bias=0.0, scale=1.0)
                    # prefill j=1 banks
                    nc.scalar.activation(pbank[S][pi][0][:], c[:, 0:512],
                                         Copy, scale=1.0)
                    nc.vector.tensor_copy(pbank[S][pi][1][:], c[:, 512:1024])
                    ust[n] = [w, c, vh, u]
                state[i] = ust

            def emit_phase_b(i):
                step, pn = units[i]
                S = i % 2
                ust = state.pop(i)
                for j in range(1, INNER):
                    last = j == INNER - 1
                    for pi, n in enumerate((pn, pn - 1)):
                        w, c, vh, u = ust[n]
                        t = kpool.tile([128, H], F16, tag="t", bufs=4,
                                       name="t")
                        for h in range(2):
                            bank = pbank[S][pi][h]
                            for m in range(4):
                                mg = h * 4 + m
                                for k in range(KH):
                                    nc.tensor.matmul(
                                        bank[:, m * 128:(m + 1) * 128],
                                        w[k][:, mg * 128:(mg + 1) * 128],
                                        u[:, k * 128:(k + 1) * 128],
                                        start=False, stop=(k == KH - 1),
                                        skip_group_check=(k == 0),
                                    )
                            nc.scalar.activation(
                                t[:, h * 512:(h + 1) * 512], bank[:], Tanh,
                                bias=0.0, scale=1.0)
                            if not last:
                                # prefill for next iteration
                                if h == 0:
                                    nc.scalar.activation(
                                        bank[:], c[:, 0:512], Copy, scale=1.0)
                                else:
                                    nc.vector.tensor_copy(
                                        bank[:], c[:, 512:1024])
                        if not last:
                            un = kpool.tile([128, H], F16, tag="u", bufs=6,
                                            name="un")
                            nc.vector.scalar_tensor_tensor(
                                un[:], u[:], 0.5, t[:], MULT, ADD)
                            ust[n][3] = un
                        else:
                            u5 = kpool.tile([128, H], F16, tag="u5", bufs=2,
                                            name="u5")
                            nc.vector.scalar_tensor_tensor(
                                u5[:], u[:], 0.5, t[:], MULT, ADD)
                            nc.vector.scalar_tensor_tensor(
                                v[n][:], u5[:], 0.25, vh[:], MULT, ADD)

            # ---- head: out^T = head_W @ v[9]^T + head_b ----
            def emit_head(S):
                nc.sync.dma_start(headWT_sb[:], headWT[:])
                nc.sync.dma_start(headB_sb[:], headB[:])
                outsb = kpool.tile([128, KO * BL], F32, tag="outsb", bufs=1)
                v9h = kpool.tile([128, H], F16, tag="v9h", bufs=1)
                nc.vector.tensor_copy(v9h[:], v[NB - 1][:])
                for m in range(KO):
                    ph = pbank[S][0][0][:, m * 128:(m + 1) * 128]
                    for k in range(KH):
                        nc.tensor.matmul(
                            ph,
                            headWT_sb[:, k * DOUT + m * 128:k * DOUT + (m + 1) * 128],
                            v9h[:, k * 128:(k + 1) * 128],
                            start=(k == 0), stop=(k == KH - 1),
                        )
                    nc.scalar.activation(
                        outsb[:, m * BL:(m + 1) * BL], ph, Ident,
                        bias=headB_sb[:, m:m + 1], scale=1.0,
                    )
                nc.sync.dma_start(outT[:], outsb[:])

            emit_phase_a(0)
            for n in range(NB - 4, -1, -1):
                nc.vector.memset(v[n][:], 0.0)
            nc.sync.dma_start(xT_sb[:], xT[:])
            nc.sync.dma_start(embWT_sb[:], embWT[:])
            nc.sync.dma_start(embB_sb[:], embB[:])
            for i in range(len(units)):
                if i + 1 < len(units):
                    emit_phase_a(i + 1)
                emit_phase_b(i)
                if i == 0:
                    emit_embed()
                if i == len(units) - 5:
                    emit_head(i % 2)

    nc.compile()
    return nc


def _tile_k(a):
    """[K, M] -> [128, (K//128)*M] laid out (k_lo, k_hi, m)."""
    K, M = a.shape
    return np.ascontiguousarray(
        a.reshape(K // 128, 128, M).transpose(1, 0, 2).reshape(128, (K // 128) * M)
    )


def kernel(**inputs) -> np.ndarray:
    x = np.asarray(inputs["x"], np.float32)
    embed_W = np.asarray(inputs["embed_W"], np.float32)
    embed_b = np.asarray(inputs["embed_b"], np.float32)
    block_W = np.asarray(inputs["block_W"], np.float32)
    block_b = np.asarray(inputs["block_b"], np.float32)
    head_W = np.asarray(inputs["head_W"], np.float32)
    head_b = np.asarray(inputs["head_b"], np.float32)
    steps = int(np.asarray(inputs["steps"]))
    with_bias = bool(np.any(block_b))

    embWT = _tile_k(embed_W.T).astype(np.float16)
    headWT = _tile_k(head_W.T).astype(np.float16)
    Wt = block_W.transpose(0, 2, 1) * np.float32(0.5)  # [NB, K=h_in, M=d_out]
    Wa = Wt.astype(np.float16)
    Wb = (2.0 * Wt - Wa.astype(np.float32)).astype(np.float16)
    Wab = np.stack(
        [
            np.stack([_tile_k(Wa[n]) for n in range(NB)]),
            np.stack([_tile_k(Wb[n]) for n in range(NB)]),
        ]
    )  # [2, NB, 128, 8*1024] f16
    embB = np.ascontiguousarray(embed_b.reshape(KH, 128).T)
    bT = np.ascontiguousarray(
        block_b.reshape(NB, KH, 128).transpose(2, 0, 1).reshape(128, NB * KH)
    )
    headB = np.ascontiguousarray(head_b.reshape(KO, 128).T)

    in_maps = []
    for ci in range(NCORES):
        xTl = _tile_k(np.ascontiguousarray(x[ci * BL:(ci + 1) * BL].T)).astype(np.float16)
        in_maps.append(
            dict(xT=xTl, embWT=embWT, embB=embB, Wab=Wab, bT=bT,
                 headWT=headWT, headB=headB)
        )

    nc = build_nc(steps, with_bias)
    res = run_bass_kernel_spmd(nc, in_maps, core_ids=list(range(NCORES)))

    out = np.empty((B, DOUT), np.float32)
    for ci in range(NCORES):
        oT = res.results[ci]["outT"]
        out[ci * BL:(ci + 1) * BL] = (
            oT.reshape(128, KO, BL).transpose(2, 1, 0).reshape(BL, DOUT)
        )
    return out
